# revision 57
# baseline (speedup 1.0000x reference)
"""Causal self-attention on 8 Trainium2 NeuronCores (pipelined v2.1).

Problem: B=2, T=2048, C=1024, 16 heads x 64 dim, fp32 in/out.

Sharding: tensor-parallel over heads x data-parallel over batch.
Each core owns one batch element (cores 0-3 -> b=0, 4-7 -> b=1) and a
group of 4 consecutive heads. Per core:
  - QKV projection for its 4 heads (qT/kT transposed, V natural)
  - causal attention for its 4 heads (scores transposed: ST[tk, tq])
  - partial output projection (its heads' rows of w_proj)
The host sums the 4 partial projections per batch and adds b_proj.

Perf structure (fp8 was tried and rejected: quantizing p or v costs ~3.5%
output error vs the 2e-2 budget — bf16 matmuls are the precision floor):
  - all matmul operands bf16: halves input DMA vs fp32, same PE rate
  - DMA issue order: per-chunk weight DMAs first (spread across queues),
    then x slab-major; wo deferred past slab 1
  - single software-pipelined schedule: QKV-projection and out-projection
    matmuls are queued as "filler" units injected one per tk iteration of
    the attention loop, so the PE never waits on the Scalar-engine exp
    chain and all phases overlap
  - output partials stored bf16 (host sums in fp32): halves output DMA

Device layouts (per core, DRAM):
  xT   [1024, 2048] bf16  x[b] transposed (channels on partitions)
  wqk  [1024, 512]  bf16  q(h0)|q(h1)|k(h0)|k(h1)|q(h2)|q(h3)|k(h2)|k(h3)
  wv   [1024, 256]  bf16  v cols of the 4 heads
  wo   [256, 1024]  bf16  w_proj rows of the 4 heads
  bqk  [4, 128]     f32   rows: pair0-q, pair0-k, pair1-q, pair1-k biases
  bv   [256]        f32   v bias of the 4 heads
  mask [128, 128]   bf16  mask[i,j] = 1 if i<=j else 0 (tk<=tq keep)
  out  [2048, 1024] bf16  partial (pre-bias) output projection

Attention per head pair (partitions 0-63 head even, 64-127 head odd):
  qT/kT [64, T] from matmul(lhsT=w_cols, rhs=xT slabs)
  ST    [tk, tq] = matmul(lhsT=kT tile, rhs=qT slab)      (K=64)
  expST = exp(0.125 * ST) on ACT, diagonal blocks masked by multiply
  yT_ext[65, tq] = matmul(lhsT=[V|1] tile, rhs=expST)     acc over tk
  yT = yT_ext[0:64] * reciprocal(yT_ext[64])              (softmax denom)
  out += matmul(lhsT=yT tiles, rhs=wo)                    (K=256)
"""

from collections import deque

import numpy as np

B, T, C = 2, 2048, 1024
NH, DH = 16, 64
NCORES = 8
HPC = 4  # heads per core
P = 128
CK = C // P  # 8 contraction tiles over channels
NT = T // P  # 16 token tiles
SLAB = 512
NSL = T // SLAB  # 4 tq slabs

_CACHE = {}


def _build_program():
    from contextlib import ExitStack

    import concourse.bacc as bacc
    import concourse.bass as bass
    import concourse.tile as tile
    from concourse import mybir

    f32 = mybir.dt.float32
    bf16 = mybir.dt.bfloat16
    AF = mybir.ActivationFunctionType

    nc = bacc.Bacc(
        "TRN2", target_bir_lowering=False, debug=False, num_devices=NCORES
    )

    xT = nc.dram_tensor("xT", [C, T], bf16, kind="ExternalInput").ap()
    wqk = nc.dram_tensor("wqk", [C, 4 * P], bf16, kind="ExternalInput").ap()
    wv = nc.dram_tensor("wv", [C, HPC * DH], bf16, kind="ExternalInput").ap()
    wo = nc.dram_tensor("wo", [HPC * DH, C], bf16, kind="ExternalInput").ap()
    bqk = nc.dram_tensor("bqk", [4, P], f32, kind="ExternalInput").ap()
    bv = nc.dram_tensor("bv", [HPC * DH], f32, kind="ExternalInput").ap()
    mask = nc.dram_tensor("mask", [P, P], bf16, kind="ExternalInput").ap()
    out = nc.dram_tensor("out", [T, C], bf16, kind="ExternalOutput").ap()

    with tile.TileContext(nc) as tc, ExitStack() as ctx:
        const = ctx.enter_context(tc.tile_pool(name="const", bufs=1))
        # PSUM budget (8 banks of [128, 2KB]):
        #   stp 2 x [128,1024] f32 (2 banks each)  = 4  score tiles
        #   psy 2 x [128, 512] f32 (1 bank each)   = 2  PV accumulators
        #   msc 2 x [128, 512] f32 (1 bank each)   = 2  QKV / outproj units
        stp = ctx.enter_context(tc.tile_pool(name="stp", bufs=2, space="PSUM"))
        psy = ctx.enter_context(tc.tile_pool(name="psy", bufs=2, space="PSUM"))
        msc = ctx.enter_context(tc.tile_pool(name="msc", bufs=2, space="PSUM"))
        expp = ctx.enter_context(tc.tile_pool(name="expp", bufs=4))
        rbp = ctx.enter_context(tc.tile_pool(name="rbp", bufs=2))
        outp = ctx.enter_context(tc.tile_pool(name="outp", bufs=6))

        x_ch = [
            const.tile([P, CK, SLAB], bf16, name=f"x_ch{c}") for c in range(NSL)
        ]
        wqk_sb = const.tile([P, CK, 4 * P], bf16, name="wqk_sb")
        wv_sb = const.tile([P, CK, HPC * DH], bf16, name="wv_sb")
        wo_sb = const.tile([P, 2, C], bf16, name="wo_sb")
        bqk_sb = const.tile([P, 4], f32, name="bqk_sb")
        bv_sb = const.tile([P, HPC, DH], f32, name="bv_sb")
        mask_sb = const.tile([P, P], bf16, name="mask_sb")
        v_sb = const.tile([P, NT, HPC, DH + 1], bf16, name="v_sb")
        qT = [const.tile([P, T], bf16, name=f"qT{p}") for p in range(2)]
        kT = [const.tile([P, T], bf16, name=f"kT{p}") for p in range(2)]
        yT = [const.tile([P, T], bf16, name=f"yT{p}") for p in range(2)]

        # --- loads: monolithic dma_starts (descriptors spread across all 16
        # queues; small per-chunk DMAs pin to one queue and serialize).
        # Order: wqk + x slab 0 gate the first compute; wo deferred. ---
        # wqk and x slab 0 gate the first compute: interleave their chunks
        # across 16 queues so chunk 0 of each lands within ~3us and the
        # first QKV psum group can start while later chunks stream in.
        wqkv_ = wqk.rearrange("(k p) n -> k p n", p=P)
        xTv = xT.rearrange("(k p) t -> p k t", p=P)
        xTc = xT.rearrange("(k p) t -> k p t", p=P)
        for k in range(CK):
            nc.sync.dma_start(out=wqk_sb[:, k, :], in_=wqkv_[k])
            nc.sync.dma_start(out=x_ch[0][:, k, :], in_=xTc[k][:, 0:SLAB])
        nc.sync.dma_start(out=bqk_sb[:], in_=bqk.rearrange("r p -> p r"))
        nc.sync.dma_start(out=wv_sb[:], in_=wv.rearrange("(k p) n -> p k n", p=P))
        bv_bcast = bass.AP(
            tensor=bv.tensor,
            offset=bv.offset,
            ap=[[0, P], *bv.rearrange("(h d) -> h d", d=DH).ap],
        )
        nc.sync.dma_start(out=bv_sb[:], in_=bv_bcast)
        nc.sync.dma_start(out=mask_sb[:], in_=mask)
        nc.sync.dma_start(out=x_ch[1][:], in_=xTv[:, :, SLAB : 2 * SLAB])
        nc.sync.dma_start(out=wo_sb[:], in_=wo.rearrange("(r p) n -> p r n", p=P))
        for c in range(2, NSL):
            nc.sync.dma_start(
                out=x_ch[c][:], in_=xTv[:, :, c * SLAB : (c + 1) * SLAB]
            )
        nc.vector.memset(v_sb[:, :, :, DH : DH + 1], 1.0)

        # ---------- work units ----------
        def b_qk_unit(s, blk, id_on_dve=True):
            """QKV projection, one q/k column block of slab s (8 matmuls)."""
            p, qk = divmod(blk, 2)
            dst = qT[p] if qk == 0 else kT[p]
            ps = msc.tile([P, SLAB], f32, name="psb", tag="msc")
            for k in range(CK):
                nc.tensor.matmul(
                    ps[:],
                    lhsT=wqk_sb[:, k, blk * P : (blk + 1) * P],
                    rhs=x_ch[s][:, k, :],
                    start=(k == 0),
                    stop=(k == CK - 1),
                )
            if id_on_dve:
                # slab-0 blocks run before attention: DVE is idle there and
                # keeping ACT clear lets C(0)'s first exp start sooner
                nc.vector.tensor_scalar_add(
                    out=dst[:, s * SLAB : (s + 1) * SLAB],
                    in0=ps[:],
                    scalar1=bqk_sb[:, blk : blk + 1],
                )
            else:
                nc.scalar.activation(
                    out=dst[:, s * SLAB : (s + 1) * SLAB],
                    in_=ps[:],
                    func=AF.Identity,
                    bias=bqk_sb[:, blk : blk + 1],
                    scale=1.0,
                )

        def b_v_unit(s, tt):
            """V projection for token tile 4s+tt (natural layout)."""
            t = 4 * s + tt
            ps = msc.tile([P, SLAB], f32, name="psv", tag="msc")
            for k in range(CK):
                nc.tensor.matmul(
                    ps[:, : HPC * DH],
                    lhsT=x_ch[s][:, k, tt * P : (tt + 1) * P],
                    rhs=wv_sb[:, k, :],
                    start=(k == 0),
                    stop=(k == CK - 1),
                )
            nc.vector.tensor_add(
                out=v_sb[:, t, :, 0:DH],
                in0=ps[:, : HPC * DH].rearrange("p (h d) -> p h d", d=DH),
                in1=bv_sb[:],
            )

        def d_unit(t, ns, use_stp=False):
            """Output projection, token tile t, column half ns.

            use_stp: draw the psum from the score pool (idle during the
            final drain) so four units pipeline instead of two.
            """
            if use_stp:
                pso = stp.tile([P, 2 * SLAB], f32, name="pp", tag="st")[:, :SLAB]
            else:
                pso = msc.tile([P, SLAB], f32, name="pso", tag="msc")
            for p in range(2):
                nc.tensor.matmul(
                    pso[:],
                    lhsT=yT[p][:, t * P : (t + 1) * P],
                    rhs=wo_sb[:, p, ns * SLAB : (ns + 1) * SLAB],
                    start=(p == 0),
                    stop=(p == 1),
                )
            ob = outp.tile([P, SLAB], bf16, name="ob", tag="ob")
            # drain units: the two psum lanes (msc/stp) are gated by their
            # own copies, so give each lane its own engine (ACT is idle in
            # the drain; Identity shares the loaded table with Exp)
            if use_stp:
                nc.scalar.add(out=ob[:], in_=pso[:], add=0.0)
            else:
                nc.vector.tensor_copy(out=ob[:], in_=pso[:])
            nc.sync.dma_start(
                out=out[t * P : (t + 1) * P, ns * SLAB : (ns + 1) * SLAB],
                in_=ob[:],
            )

        fill = deque()

        def pop_fill():
            if fill:
                fill.popleft()()

        # ---------- attention for one head pair, one tq slab ----------
        def attn_pair(p, s):
            psyt = [
                psy.tile([P, SLAB], f32, name=f"psy{hp}", tag="psy")
                for hp in range(2)
            ]
            ntk = 4 * s + 4  # tk tiles 0 .. 4s+3 (causal)

            def off_of(tk):
                d = tk - 4 * s
                return d * P if d > 0 else 0

            def st_pair(tk):
                off = off_of(tk)
                pp = stp.tile([P, 2 * SLAB], f32, name="pp", tag="st")
                for hp in range(2):
                    nc.tensor.matmul(
                        pp[:, hp * SLAB + off : (hp + 1) * SLAB],
                        lhsT=kT[p][hp * DH : (hp + 1) * DH, tk * P : (tk + 1) * P],
                        rhs=qT[p][hp * DH : (hp + 1) * DH, s * SLAB + off : (s + 1) * SLAB],
                        start=True,
                        stop=True,
                    )
                return pp

            def do_exp(tk):
                off = off_of(tk)
                pp = pend.pop(tk)
                ex = expp.tile([P, 2 * SLAB], bf16, name="ex", tag="ex")
                ppv = pp[:].rearrange("q (h n) -> q h n", h=2)[:, :, off:]
                exv = ex[:].rearrange("q (h n) -> q h n", h=2)[:, :, off:]
                nc.scalar.activation(
                    out=exv,
                    in_=ppv,
                    func=AF.Exp,
                    scale=float(1.0 / np.sqrt(DH)),
                )
                if tk - 4 * s >= 0:
                    for hp in range(2):
                        nc.vector.tensor_mul(
                            out=ex[:, hp * SLAB + off : hp * SLAB + off + P],
                            in0=ex[:, hp * SLAB + off : hp * SLAB + off + P],
                            in1=mask_sb[:],
                        )
                return ex

            # Pace fillers evenly across the block: pair 0 consumes half the
            # queue, pair 1 the rest, so the ACT-bound late iterations of a
            # block still have PE filler work instead of draining the queue
            # up front.
            share = len(fill) if p == 1 else (len(fill) + 1) // 2
            popped = 0
            pend = {0: st_pair(0)}
            if ntk > 1:
                pend[1] = st_pair(1)
            exd = {0: do_exp(0)}
            for tk in range(ntk):
                off = off_of(tk)
                if tk + 2 < ntk:
                    pend[tk + 2] = st_pair(tk + 2)
                if tk + 1 < ntk:
                    exd[tk + 1] = do_exp(tk + 1)
                target = (share * (tk + 1) + ntk - 1) // ntk
                while popped < target and fill:
                    fill.popleft()()
                    popped += 1
                ex = exd.pop(tk)
                for hp in range(2):
                    nc.tensor.matmul(
                        psyt[hp][0 : DH + 1, off:],
                        lhsT=v_sb[:, tk, 2 * p + hp, :],
                        rhs=ex[:, hp * SLAB + off : (hp + 1) * SLAB],
                        start=(tk == 0),
                        stop=(tk == ntk - 1),
                    )
            # Evacuate each PV accumulator to SBUF in one copy (same DVE cost
            # as copying just the denominator row — cost scales with free
            # size, not partitions) so the PSUM bank frees ~0.7us after the
            # last PV instead of after the whole normalize chain; the next
            # pair-block's first PV reuses it without stalling.
            # NOTE: reciprocal_approx_fast silently corrupts on HW when its
            # input is PSUM or a non-partition-0 slice (sim models both
            # fine) — it must read the partition-0 SBUF sm copy.
            if p == 1 and s == NSL - 1:
                # Final chain is on the critical path to the drain: stage
                # both heads' ops so DVE doesn't head-of-line block on the
                # gpsimd broadcast between rec and mul.
                sms, recs, rbs = [], [], []
                for hp in range(2):
                    sm = rbp.tile([1, SLAB], f32, name="sm", tag="sm")
                    nc.vector.tensor_copy(
                        out=sm[:], in_=psyt[hp][DH : DH + 1, :]
                    )
                    sms.append(sm)
                for hp in range(2):
                    rec = rbp.tile([1, SLAB], f32, name="rec", tag="rec")
                    nc.vector.reciprocal_approx_fast(out=rec[:], in_=sms[hp][:])
                    recs.append(rec)
                for hp in range(2):
                    rb = rbp.tile([DH, SLAB], f32, name="rb", tag="rb")
                    nc.gpsimd.partition_broadcast(out_ap=rb[:], in_ap=recs[hp][:])
                    rbs.append(rb)
                for hp in range(2):
                    nc.vector.tensor_mul(
                        out=yT[p][hp * DH : (hp + 1) * DH, s * SLAB : (s + 1) * SLAB],
                        in0=psyt[hp][0:DH, :],
                        in1=rbs[hp][:],
                    )
            else:
                for hp in range(2):
                    sm = rbp.tile([1, SLAB], f32, name="sm", tag="sm")
                    nc.vector.tensor_copy(
                        out=sm[:], in_=psyt[hp][DH : DH + 1, :]
                    )
                    rec = rbp.tile([1, SLAB], f32, name="rec", tag="rec")
                    nc.vector.reciprocal_approx_fast(out=rec[:], in_=sm[:])
                    rb = rbp.tile([DH, SLAB], f32, name="rb", tag="rb")
                    nc.gpsimd.partition_broadcast(out_ap=rb[:], in_ap=rec[:])
                    nc.vector.tensor_mul(
                        out=yT[p][hp * DH : (hp + 1) * DH, s * SLAB : (s + 1) * SLAB],
                        in0=psyt[hp][0:DH, :],
                        in1=rb[:],
                    )

        # ---------- schedule ----------
        # Slab-0 q/k runs chunk-major across all 4 column blocks (4 open
        # psum groups: 2 msc banks + 2 borrowed from the idle score pool)
        # so each arriving x/wqk chunk feeds ~850ns of PE work against the
        # ~640ns DMA cadence — the PE never starves during the load.
        ps_l = [
            msc.tile([P, SLAB], f32, name="psb0", tag="msc"),
            msc.tile([P, SLAB], f32, name="psb1", tag="msc"),
            stp.tile([P, 2 * SLAB], f32, name="pp", tag="st")[:, :SLAB],
            stp.tile([P, 2 * SLAB], f32, name="pp", tag="st")[:, :SLAB],
        ]
        for k in range(CK):
            for blk in range(4):
                nc.tensor.matmul(
                    ps_l[blk][:],
                    lhsT=wqk_sb[:, k, blk * P : (blk + 1) * P],
                    rhs=x_ch[0][:, k, :],
                    start=(k == 0),
                    stop=(k == CK - 1),
                )
        for blk in range(4):
            p_, qk = divmod(blk, 2)
            dst = qT[p_] if qk == 0 else kT[p_]
            nc.vector.tensor_scalar_add(
                out=dst[:, 0:SLAB],
                in0=ps_l[blk][:],
                scalar1=bqk_sb[:, blk : blk + 1],
            )
        for tt in range(4):
            b_v_unit(0, tt)
        # Filler distribution: the exp-heaviest attention block C(3) is
        # ACT-bound by ~12us, so defer everything legal into it: B(3)'s
        # k/v units (C(3) STs only touch kT slab 3 from tk tile 12, by
        # which time they've been popped) and both D(1)/D(2).  Only the
        # q-slab-3 blocks must fully precede C(3).
        for s in range(NSL):
            if s + 1 < 3:
                for blk in range(4):
                    fill.append(lambda s1=s + 1, blk=blk: b_qk_unit(s1, blk))
                for tt in range(4):
                    fill.append(lambda s1=s + 1, tt=tt: b_v_unit(s1, tt))
            elif s + 1 == 3:
                for blk in (0, 2):
                    fill.append(lambda blk=blk: b_qk_unit(3, blk))
                for t in range(2, 4):  # back half of D(0) pads C(2)
                    for ns in range(2):
                        fill.append(lambda t=t, ns=ns: d_unit(t, ns))
            else:  # s == 3: k/v of slab 3 first, then outproj of slabs 1-2
                for blk in (1, 3):
                    fill.append(lambda blk=blk: b_qk_unit(3, blk))
                for tt in range(4):
                    fill.append(lambda tt=tt: b_v_unit(3, tt))
                for t in range(4, 10):
                    for ns in range(2):
                        fill.append(lambda t=t, ns=ns: d_unit(t, ns))
            if s == 1:
                for t in range(2):
                    for ns in range(2):
                        fill.append(lambda t=t, ns=ns: d_unit(t, ns))
            attn_pair(0, s)
            attn_pair(1, s)
        while fill:
            pop_fill()
        # slab-2 leftovers run during the final normalize chain (they don't
        # depend on it), then slab 3 once yT is complete
        for t in (10, 11):
            for ns in range(2):
                d_unit(t, ns, use_stp=(ns == 1))
        for tt in range(4):
            for ns in range(2):
                d_unit(12 + tt, ns, use_stp=(ns == 1))

    nc.compile()
    return nc


def get_program():
    if "nc" not in _CACHE:
        _CACHE["nc"] = _build_program()
    return _CACHE["nc"]


def make_core_inputs(x, w_attn, b_attn, w_proj, core):
    """Host-side shard preparation for one core."""
    import ml_dtypes

    bf16 = ml_dtypes.bfloat16
    b = core // 4
    g = core % 4
    heads = [4 * g + i for i in range(HPC)]

    xT = np.ascontiguousarray(np.asarray(x[b], np.float32).T.astype(bf16))

    def qcols(h):
        return w_attn[:, h * DH : (h + 1) * DH]

    def kcols(h):
        return w_attn[:, C + h * DH : C + (h + 1) * DH]

    def vcols(h):
        return w_attn[:, 2 * C + h * DH : 2 * C + (h + 1) * DH]

    h0, h1, h2, h3 = heads
    wqk = np.ascontiguousarray(
        np.concatenate(
            [qcols(h0), qcols(h1), kcols(h0), kcols(h1),
             qcols(h2), qcols(h3), kcols(h2), kcols(h3)],
            axis=1,
        ).astype(bf16)
    )
    wv = np.ascontiguousarray(
        np.concatenate([vcols(h) for h in heads], axis=1).astype(bf16)
    )
    bqk = np.stack(
        [
            np.concatenate([b_attn[h0 * DH : (h0 + 1) * DH], b_attn[h1 * DH : (h1 + 1) * DH]]),
            np.concatenate([b_attn[C + h0 * DH : C + (h0 + 1) * DH], b_attn[C + h1 * DH : C + (h1 + 1) * DH]]),
            np.concatenate([b_attn[h2 * DH : (h2 + 1) * DH], b_attn[h3 * DH : (h3 + 1) * DH]]),
            np.concatenate([b_attn[C + h2 * DH : C + (h2 + 1) * DH], b_attn[C + h3 * DH : C + (h3 + 1) * DH]]),
        ]
    ).astype(np.float32)
    bv = np.concatenate(
        [b_attn[2 * C + h * DH : 2 * C + (h + 1) * DH] for h in heads]
    ).astype(np.float32)
    wo = np.ascontiguousarray(
        w_proj[heads[0] * DH : (heads[-1] + 1) * DH, :].astype(bf16)
    )
    mask = np.triu(np.ones((P, P))).astype(bf16)
    return {
        "xT": xT,
        "wqk": wqk,
        "wv": wv,
        "wo": wo,
        "bqk": np.ascontiguousarray(bqk),
        "bv": np.ascontiguousarray(bv),
        "mask": mask,
    }


def kernel(x, w_attn, b_attn, w_proj, b_proj):
    from concourse.bass_utils import run_bass_kernel_spmd

    x = np.asarray(x, np.float32)
    w_attn = np.asarray(w_attn, np.float32)
    b_attn = np.asarray(b_attn, np.float32)
    w_proj = np.asarray(w_proj, np.float32)
    b_proj = np.asarray(b_proj, np.float32)

    nc = get_program()
    in_maps = [
        make_core_inputs(x, w_attn, b_attn, w_proj, core) for core in range(NCORES)
    ]
    res = run_bass_kernel_spmd(nc, in_maps, core_ids=list(range(NCORES)))
    outs = [np.asarray(m["out"], np.float32) for m in res.results]

    y = np.empty((B, T, C), np.float32)
    for b in range(B):
        y[b] = outs[4 * b] + outs[4 * b + 1] + outs[4 * b + 2] + outs[4 * b + 3]
        y[b] += b_proj[None, :]
    return y


# revision 58
# speedup vs baseline: 1.0111x; 1.0111x over previous
"""Causal self-attention on 8 Trainium2 NeuronCores (pipelined v2.1).

Problem: B=2, T=2048, C=1024, 16 heads x 64 dim, fp32 in/out.

Sharding: tensor-parallel over heads x data-parallel over batch.
Each core owns one batch element (cores 0-3 -> b=0, 4-7 -> b=1) and a
group of 4 consecutive heads. Per core:
  - QKV projection for its 4 heads (qT/kT transposed, V natural)
  - causal attention for its 4 heads (scores transposed: ST[tk, tq])
  - partial output projection (its heads' rows of w_proj)
The host sums the 4 partial projections per batch and adds b_proj.

Perf structure (fp8 was tried and rejected: quantizing p or v costs ~3.5%
output error vs the 2e-2 budget — bf16 matmuls are the precision floor):
  - all matmul operands bf16: halves input DMA vs fp32, same PE rate
  - DMA issue order: per-chunk weight DMAs first (spread across queues),
    then x slab-major; wo deferred past slab 1
  - single software-pipelined schedule: QKV-projection and out-projection
    matmuls are queued as "filler" units injected one per tk iteration of
    the attention loop, so the PE never waits on the Scalar-engine exp
    chain and all phases overlap
  - output partials stored bf16 (host sums in fp32): halves output DMA

Device layouts (per core, DRAM):
  xT   [1024, 2048] bf16  x[b] transposed (channels on partitions)
  wqk  [1024, 512]  bf16  q(h0)|q(h1)|k(h0)|k(h1)|q(h2)|q(h3)|k(h2)|k(h3)
  wv   [1024, 256]  bf16  v cols of the 4 heads
  wo   [256, 1024]  bf16  w_proj rows of the 4 heads
  bqk  [4, 128]     f32   rows: pair0-q, pair0-k, pair1-q, pair1-k biases
  bv   [256]        f32   v bias of the 4 heads
  mask [128, 128]   bf16  mask[i,j] = 1 if i<=j else 0 (tk<=tq keep)
  out  [2048, 1024] bf16  partial (pre-bias) output projection

Attention per head pair (partitions 0-63 head even, 64-127 head odd):
  qT/kT [64, T] from matmul(lhsT=w_cols, rhs=xT slabs)
  ST    [tk, tq] = matmul(lhsT=kT tile, rhs=qT slab)      (K=64)
  expST = exp(0.125 * ST) on ACT, diagonal blocks masked by multiply
  yT_ext[65, tq] = matmul(lhsT=[V|1] tile, rhs=expST)     acc over tk
  yT = yT_ext[0:64] * reciprocal(yT_ext[64])              (softmax denom)
  out += matmul(lhsT=yT tiles, rhs=wo)                    (K=256)
"""

from collections import deque

import numpy as np

B, T, C = 2, 2048, 1024
NH, DH = 16, 64
NCORES = 8
HPC = 4  # heads per core
P = 128
CK = C // P  # 8 contraction tiles over channels
NT = T // P  # 16 token tiles
SLAB = 512
NSL = T // SLAB  # 4 tq slabs

_CACHE = {}


def _build_program():
    from contextlib import ExitStack

    import concourse.bacc as bacc
    import concourse.bass as bass
    import concourse.tile as tile
    from concourse import mybir

    f32 = mybir.dt.float32
    bf16 = mybir.dt.bfloat16
    AF = mybir.ActivationFunctionType

    nc = bacc.Bacc(
        "TRN2", target_bir_lowering=False, debug=False, num_devices=NCORES
    )

    xT = nc.dram_tensor("xT", [C, T], bf16, kind="ExternalInput").ap()
    wqk = nc.dram_tensor("wqk", [C, 4 * P], bf16, kind="ExternalInput").ap()
    wv = nc.dram_tensor("wv", [C, HPC * DH], bf16, kind="ExternalInput").ap()
    wo = nc.dram_tensor("wo", [HPC * DH, C], bf16, kind="ExternalInput").ap()
    bqk = nc.dram_tensor("bqk", [4, P], f32, kind="ExternalInput").ap()
    bv = nc.dram_tensor("bv", [HPC * DH], f32, kind="ExternalInput").ap()
    mask = nc.dram_tensor("mask", [P, P], bf16, kind="ExternalInput").ap()
    out = nc.dram_tensor("out", [T, C], bf16, kind="ExternalOutput").ap()

    with tile.TileContext(nc) as tc, ExitStack() as ctx:
        const = ctx.enter_context(tc.tile_pool(name="const", bufs=1))
        # PSUM budget (8 banks of [128, 2KB]):
        #   stp 2 x [128,1024] f32 (2 banks each)  = 4  score tiles
        #   psy 2 x [128, 512] f32 (1 bank each)   = 2  PV accumulators
        #   msc 2 x [128, 512] f32 (1 bank each)   = 2  QKV / outproj units
        stp = ctx.enter_context(tc.tile_pool(name="stp", bufs=2, space="PSUM"))
        psy = ctx.enter_context(tc.tile_pool(name="psy", bufs=2, space="PSUM"))
        msc = ctx.enter_context(tc.tile_pool(name="msc", bufs=2, space="PSUM"))
        expp = ctx.enter_context(tc.tile_pool(name="expp", bufs=4))
        rbp = ctx.enter_context(tc.tile_pool(name="rbp", bufs=2))
        outp = ctx.enter_context(tc.tile_pool(name="outp", bufs=6))

        x_ch = [
            const.tile([P, CK, SLAB], bf16, name=f"x_ch{c}") for c in range(NSL)
        ]
        wqk_sb = const.tile([P, CK, 4 * P], bf16, name="wqk_sb")
        wv_sb = const.tile([P, CK, HPC * DH], bf16, name="wv_sb")
        wo_sb = const.tile([P, 2, C], bf16, name="wo_sb")
        bqk_sb = const.tile([P, 4], f32, name="bqk_sb")
        bv_sb = const.tile([P, HPC, DH], f32, name="bv_sb")
        mask_sb = const.tile([P, P], bf16, name="mask_sb")
        v_sb = const.tile([P, NT, HPC, DH + 1], bf16, name="v_sb")
        qT = [const.tile([P, T], bf16, name=f"qT{p}") for p in range(2)]
        kT = [const.tile([P, T], bf16, name=f"kT{p}") for p in range(2)]
        yT = [const.tile([P, T], bf16, name=f"yT{p}") for p in range(2)]

        # --- loads: monolithic dma_starts (descriptors spread across all 16
        # queues; small per-chunk DMAs pin to one queue and serialize).
        # Order: wqk + x slab 0 gate the first compute; wo deferred. ---
        # wqk and x slab 0 gate the first compute: interleave their chunks
        # across 16 queues so chunk 0 of each lands within ~3us and the
        # first QKV psum group can start while later chunks stream in.
        wqkv_ = wqk.rearrange("(k p) n -> k p n", p=P)
        xTv = xT.rearrange("(k p) t -> p k t", p=P)
        xTc = xT.rearrange("(k p) t -> k p t", p=P)
        for k in range(CK):
            nc.sync.dma_start(out=wqk_sb[:, k, :], in_=wqkv_[k])
            nc.sync.dma_start(out=x_ch[0][:, k, :], in_=xTc[k][:, 0:SLAB])
        nc.sync.dma_start(out=bqk_sb[:], in_=bqk.rearrange("r p -> p r"))
        nc.sync.dma_start(out=wv_sb[:], in_=wv.rearrange("(k p) n -> p k n", p=P))
        bv_bcast = bass.AP(
            tensor=bv.tensor,
            offset=bv.offset,
            ap=[[0, P], *bv.rearrange("(h d) -> h d", d=DH).ap],
        )
        nc.sync.dma_start(out=bv_sb[:], in_=bv_bcast)
        nc.sync.dma_start(out=mask_sb[:], in_=mask)
        nc.sync.dma_start(out=x_ch[1][:], in_=xTv[:, :, SLAB : 2 * SLAB])
        nc.sync.dma_start(out=wo_sb[:], in_=wo.rearrange("(r p) n -> p r n", p=P))
        for c in range(2, NSL):
            nc.sync.dma_start(
                out=x_ch[c][:], in_=xTv[:, :, c * SLAB : (c + 1) * SLAB]
            )
        nc.vector.memset(v_sb[:, :, :, DH : DH + 1], 1.0)

        # ---------- work units ----------
        def b_qk_unit(s, blk, id_on_dve=False):
            """QKV projection, one q/k column block of slab s (8 matmuls)."""
            p, qk = divmod(blk, 2)
            dst = qT[p] if qk == 0 else kT[p]
            ps = msc.tile([P, SLAB], f32, name="psb", tag="msc")
            for k in range(CK):
                nc.tensor.matmul(
                    ps[:],
                    lhsT=wqk_sb[:, k, blk * P : (blk + 1) * P],
                    rhs=x_ch[s][:, k, :],
                    start=(k == 0),
                    stop=(k == CK - 1),
                )
            if id_on_dve:
                # slab-0 blocks run before attention: DVE is idle there and
                # keeping ACT clear lets C(0)'s first exp start sooner
                nc.vector.tensor_scalar_add(
                    out=dst[:, s * SLAB : (s + 1) * SLAB],
                    in0=ps[:],
                    scalar1=bqk_sb[:, blk : blk + 1],
                )
            else:
                nc.scalar.activation(
                    out=dst[:, s * SLAB : (s + 1) * SLAB],
                    in_=ps[:],
                    func=AF.Identity,
                    bias=bqk_sb[:, blk : blk + 1],
                    scale=1.0,
                )

        def b_v_unit(s, tt):
            """V projection for token tile 4s+tt (natural layout)."""
            t = 4 * s + tt
            ps = msc.tile([P, SLAB], f32, name="psv", tag="msc")
            for k in range(CK):
                nc.tensor.matmul(
                    ps[:, : HPC * DH],
                    lhsT=x_ch[s][:, k, tt * P : (tt + 1) * P],
                    rhs=wv_sb[:, k, :],
                    start=(k == 0),
                    stop=(k == CK - 1),
                )
            nc.vector.tensor_add(
                out=v_sb[:, t, :, 0:DH],
                in0=ps[:, : HPC * DH].rearrange("p (h d) -> p h d", d=DH),
                in1=bv_sb[:],
            )

        def d_unit(t, ns, use_stp=False):
            """Output projection, token tile t, column half ns.

            use_stp: draw the psum from the score pool (idle during the
            final drain) so four units pipeline instead of two.
            """
            if use_stp:
                pso = stp.tile([P, 2 * SLAB], f32, name="pp", tag="st")[:, :SLAB]
            else:
                pso = msc.tile([P, SLAB], f32, name="pso", tag="msc")
            for p in range(2):
                nc.tensor.matmul(
                    pso[:],
                    lhsT=yT[p][:, t * P : (t + 1) * P],
                    rhs=wo_sb[:, p, ns * SLAB : (ns + 1) * SLAB],
                    start=(p == 0),
                    stop=(p == 1),
                )
            ob = outp.tile([P, SLAB], bf16, name="ob", tag="ob")
            # drain units: the two psum lanes (msc/stp) are gated by their
            # own copies, so give each lane its own engine (ACT is idle in
            # the drain; Identity shares the loaded table with Exp)
            if use_stp:
                nc.scalar.add(out=ob[:], in_=pso[:], add=0.0)
            else:
                nc.vector.tensor_copy(out=ob[:], in_=pso[:])
            nc.sync.dma_start(
                out=out[t * P : (t + 1) * P, ns * SLAB : (ns + 1) * SLAB],
                in_=ob[:],
            )

        fill = deque()

        def pop_fill():
            if fill:
                fill.popleft()()

        # ---------- attention for one head pair, one tq slab ----------
        def attn_pair(p, s):
            psyt = [
                psy.tile([P, SLAB], f32, name=f"psy{hp}", tag="psy")
                for hp in range(2)
            ]
            ntk = 4 * s + 4  # tk tiles 0 .. 4s+3 (causal)

            def off_of(tk):
                d = tk - 4 * s
                return d * P if d > 0 else 0

            def st_pair(tk):
                off = off_of(tk)
                pp = stp.tile([P, 2 * SLAB], f32, name="pp", tag="st")
                for hp in range(2):
                    nc.tensor.matmul(
                        pp[:, hp * SLAB + off : (hp + 1) * SLAB],
                        lhsT=kT[p][hp * DH : (hp + 1) * DH, tk * P : (tk + 1) * P],
                        rhs=qT[p][hp * DH : (hp + 1) * DH, s * SLAB + off : (s + 1) * SLAB],
                        start=True,
                        stop=True,
                    )
                return pp

            def do_exp(tk):
                off = off_of(tk)
                pp = pend.pop(tk)
                ex = expp.tile([P, 2 * SLAB], bf16, name="ex", tag="ex")
                ppv = pp[:].rearrange("q (h n) -> q h n", h=2)[:, :, off:]
                exv = ex[:].rearrange("q (h n) -> q h n", h=2)[:, :, off:]
                nc.scalar.activation(
                    out=exv,
                    in_=ppv,
                    func=AF.Exp,
                    scale=float(1.0 / np.sqrt(DH)),
                )
                if tk - 4 * s >= 0:
                    for hp in range(2):
                        nc.vector.tensor_mul(
                            out=ex[:, hp * SLAB + off : hp * SLAB + off + P],
                            in0=ex[:, hp * SLAB + off : hp * SLAB + off + P],
                            in1=mask_sb[:],
                        )
                return ex

            # Pace fillers evenly across the block: pair 0 consumes half the
            # queue, pair 1 the rest, so the ACT-bound late iterations of a
            # block still have PE filler work instead of draining the queue
            # up front.
            share = len(fill) if p == 1 else (len(fill) + 1) // 2
            popped = 0
            pend = {0: st_pair(0)}
            if ntk > 1:
                pend[1] = st_pair(1)
            exd = {0: do_exp(0)}
            for tk in range(ntk):
                off = off_of(tk)
                if tk + 2 < ntk:
                    pend[tk + 2] = st_pair(tk + 2)
                if tk + 1 < ntk:
                    exd[tk + 1] = do_exp(tk + 1)
                target = (share * (tk + 1) + ntk - 1) // ntk
                while popped < target and fill:
                    fill.popleft()()
                    popped += 1
                ex = exd.pop(tk)
                for hp in range(2):
                    nc.tensor.matmul(
                        psyt[hp][0 : DH + 1, off:],
                        lhsT=v_sb[:, tk, 2 * p + hp, :],
                        rhs=ex[:, hp * SLAB + off : (hp + 1) * SLAB],
                        start=(tk == 0),
                        stop=(tk == ntk - 1),
                    )
            # Evacuate each PV accumulator to SBUF in one copy (same DVE cost
            # as copying just the denominator row — cost scales with free
            # size, not partitions) so the PSUM bank frees ~0.7us after the
            # last PV instead of after the whole normalize chain; the next
            # pair-block's first PV reuses it without stalling.
            # NOTE: reciprocal_approx_fast silently corrupts on HW when its
            # input is PSUM or a non-partition-0 slice (sim models both
            # fine) — it must read the partition-0 SBUF sm copy.
            if p == 1 and s == NSL - 1:
                # Final chain is on the critical path to the drain: stage
                # both heads' ops so DVE doesn't head-of-line block on the
                # gpsimd broadcast between rec and mul.
                sms, recs, rbs = [], [], []
                for hp in range(2):
                    sm = rbp.tile([1, SLAB], f32, name="sm", tag="sm")
                    nc.vector.tensor_copy(
                        out=sm[:], in_=psyt[hp][DH : DH + 1, :]
                    )
                    sms.append(sm)
                for hp in range(2):
                    rec = rbp.tile([1, SLAB], f32, name="rec", tag="rec")
                    nc.vector.reciprocal_approx_fast(out=rec[:], in_=sms[hp][:])
                    recs.append(rec)
                for hp in range(2):
                    rb = rbp.tile([DH, SLAB], f32, name="rb", tag="rb")
                    nc.gpsimd.partition_broadcast(out_ap=rb[:], in_ap=recs[hp][:])
                    rbs.append(rb)
                for hp in range(2):
                    nc.vector.tensor_mul(
                        out=yT[p][hp * DH : (hp + 1) * DH, s * SLAB : (s + 1) * SLAB],
                        in0=psyt[hp][0:DH, :],
                        in1=rbs[hp][:],
                    )
            else:
                for hp in range(2):
                    sm = rbp.tile([1, SLAB], f32, name="sm", tag="sm")
                    nc.vector.tensor_copy(
                        out=sm[:], in_=psyt[hp][DH : DH + 1, :]
                    )
                    rec = rbp.tile([1, SLAB], f32, name="rec", tag="rec")
                    nc.vector.reciprocal_approx_fast(out=rec[:], in_=sm[:])
                    rb = rbp.tile([DH, SLAB], f32, name="rb", tag="rb")
                    nc.gpsimd.partition_broadcast(out_ap=rb[:], in_ap=rec[:])
                    nc.vector.tensor_mul(
                        out=yT[p][hp * DH : (hp + 1) * DH, s * SLAB : (s + 1) * SLAB],
                        in0=psyt[hp][0:DH, :],
                        in1=rb[:],
                    )

        # ---------- schedule ----------
        # Slab-0 q/k runs chunk-major across all 4 column blocks (4 open
        # psum groups: 2 msc banks + 2 borrowed from the idle score pool)
        # so each arriving x/wqk chunk feeds ~850ns of PE work against the
        # ~640ns DMA cadence — the PE never starves during the load.
        ps_l = [
            msc.tile([P, SLAB], f32, name="psb0", tag="msc"),
            msc.tile([P, SLAB], f32, name="psb1", tag="msc"),
            stp.tile([P, 2 * SLAB], f32, name="pp", tag="st")[:, :SLAB],
            stp.tile([P, 2 * SLAB], f32, name="pp", tag="st")[:, :SLAB],
        ]
        for k in range(CK):
            for blk in range(4):
                nc.tensor.matmul(
                    ps_l[blk][:],
                    lhsT=wqk_sb[:, k, blk * P : (blk + 1) * P],
                    rhs=x_ch[0][:, k, :],
                    start=(k == 0),
                    stop=(k == CK - 1),
                )
        for blk in range(4):
            p_, qk = divmod(blk, 2)
            dst = qT[p_] if qk == 0 else kT[p_]
            nc.vector.tensor_scalar_add(
                out=dst[:, 0:SLAB],
                in0=ps_l[blk][:],
                scalar1=bqk_sb[:, blk : blk + 1],
            )
        for tt in range(4):
            b_v_unit(0, tt)
        # Filler distribution: the exp-heaviest attention block C(3) is
        # ACT-bound by ~12us, so defer everything legal into it: B(3)'s
        # k/v units (C(3) STs only touch kT slab 3 from tk tile 12, by
        # which time they've been popped) and both D(1)/D(2).  Only the
        # q-slab-3 blocks must fully precede C(3).
        for s in range(NSL):
            if s + 1 < 3:
                for blk in range(4):
                    fill.append(lambda s1=s + 1, blk=blk: b_qk_unit(s1, blk))
                for tt in range(4):
                    fill.append(lambda s1=s + 1, tt=tt: b_v_unit(s1, tt))
            elif s + 1 == 3:
                for blk in (0, 2):
                    fill.append(lambda blk=blk: b_qk_unit(3, blk))
                for t in range(2, 4):  # back half of D(0) pads C(2)
                    for ns in range(2):
                        fill.append(lambda t=t, ns=ns: d_unit(t, ns))
            else:  # s == 3: k/v of slab 3 first, then outproj of slabs 1-2
                for blk in (1, 3):
                    fill.append(lambda blk=blk: b_qk_unit(3, blk))
                for tt in range(4):
                    fill.append(lambda tt=tt: b_v_unit(3, tt))
                for t in range(4, 10):
                    for ns in range(2):
                        fill.append(lambda t=t, ns=ns: d_unit(t, ns))
            if s == 1:
                for t in range(2):
                    for ns in range(2):
                        fill.append(lambda t=t, ns=ns: d_unit(t, ns))
            attn_pair(0, s)
            attn_pair(1, s)
        while fill:
            pop_fill()
        # slab-2 leftovers run during the final normalize chain (they don't
        # depend on it), then slab 3 once yT is complete
        for t in (10, 11):
            for ns in range(2):
                d_unit(t, ns, use_stp=(ns == 1))
        for tt in range(4):
            for ns in range(2):
                d_unit(12 + tt, ns, use_stp=(ns == 1))

    nc.compile()
    return nc


def get_program():
    if "nc" not in _CACHE:
        _CACHE["nc"] = _build_program()
    return _CACHE["nc"]


def make_core_inputs(x, w_attn, b_attn, w_proj, core):
    """Host-side shard preparation for one core."""
    import ml_dtypes

    bf16 = ml_dtypes.bfloat16
    b = core // 4
    g = core % 4
    heads = [4 * g + i for i in range(HPC)]

    xT = np.ascontiguousarray(np.asarray(x[b], np.float32).T.astype(bf16))

    def qcols(h):
        return w_attn[:, h * DH : (h + 1) * DH]

    def kcols(h):
        return w_attn[:, C + h * DH : C + (h + 1) * DH]

    def vcols(h):
        return w_attn[:, 2 * C + h * DH : 2 * C + (h + 1) * DH]

    h0, h1, h2, h3 = heads
    wqk = np.ascontiguousarray(
        np.concatenate(
            [qcols(h0), qcols(h1), kcols(h0), kcols(h1),
             qcols(h2), qcols(h3), kcols(h2), kcols(h3)],
            axis=1,
        ).astype(bf16)
    )
    wv = np.ascontiguousarray(
        np.concatenate([vcols(h) for h in heads], axis=1).astype(bf16)
    )
    bqk = np.stack(
        [
            np.concatenate([b_attn[h0 * DH : (h0 + 1) * DH], b_attn[h1 * DH : (h1 + 1) * DH]]),
            np.concatenate([b_attn[C + h0 * DH : C + (h0 + 1) * DH], b_attn[C + h1 * DH : C + (h1 + 1) * DH]]),
            np.concatenate([b_attn[h2 * DH : (h2 + 1) * DH], b_attn[h3 * DH : (h3 + 1) * DH]]),
            np.concatenate([b_attn[C + h2 * DH : C + (h2 + 1) * DH], b_attn[C + h3 * DH : C + (h3 + 1) * DH]]),
        ]
    ).astype(np.float32)
    bv = np.concatenate(
        [b_attn[2 * C + h * DH : 2 * C + (h + 1) * DH] for h in heads]
    ).astype(np.float32)
    wo = np.ascontiguousarray(
        w_proj[heads[0] * DH : (heads[-1] + 1) * DH, :].astype(bf16)
    )
    mask = np.triu(np.ones((P, P))).astype(bf16)
    return {
        "xT": xT,
        "wqk": wqk,
        "wv": wv,
        "wo": wo,
        "bqk": np.ascontiguousarray(bqk),
        "bv": np.ascontiguousarray(bv),
        "mask": mask,
    }


def kernel(x, w_attn, b_attn, w_proj, b_proj):
    from concourse.bass_utils import run_bass_kernel_spmd

    x = np.asarray(x, np.float32)
    w_attn = np.asarray(w_attn, np.float32)
    b_attn = np.asarray(b_attn, np.float32)
    w_proj = np.asarray(w_proj, np.float32)
    b_proj = np.asarray(b_proj, np.float32)

    nc = get_program()
    in_maps = [
        make_core_inputs(x, w_attn, b_attn, w_proj, core) for core in range(NCORES)
    ]
    res = run_bass_kernel_spmd(nc, in_maps, core_ids=list(range(NCORES)))
    outs = [np.asarray(m["out"], np.float32) for m in res.results]

    y = np.empty((B, T, C), np.float32)
    for b in range(B):
        y[b] = outs[4 * b] + outs[4 * b + 1] + outs[4 * b + 2] + outs[4 * b + 3]
        y[b] += b_proj[None, :]
    return y
